# revision 19
# baseline (speedup 1.0000x reference)
"""Trainium2 Bass kernel for nn_AttentionMechanism_21646635172225.

Reference computation (per batch element n):
    q   = transpose(x[n], (T,C,H,W)).reshape(T, C*H*W)      # x[n]: (C,T,H,W)
    E   = q @ q.T                                            # (T, T)
    A   = softmax(E, axis=-1)
    out = alpha * (A @ q) + q          -> reshape/transpose back to (C,T,H,W)

Sharding: data-parallel over batch N=8 across the 8 NeuronCores (one batch
element per core), alpha replicated.

Per-core dataflow (C=128 on partitions, free axis = t*784 + hw):
  Phase 1 (per hw-striped chunk): DMA x chunk -> SBUF (XNQ); ScalarE casts it
    to bf16 (XNbf); TensorE accumulates E = sum_j A_j^T A_j (A_j = x[:,:,j] as
    a [128,32] strided tile) into one PSUM tile; VectorE 32x32 block-transposes
    the chunk into the "folded t-major" layout qt[32g+t, ...] = q[t, c, hw].
    The transpose of slot m writes into slot m-1's (dead) region of XNQ
    (slot 0 into a spare tail region), so no extra full-size buffer exists.
  Softmax: E replicated x4 partition groups via a stacked-identity matmul,
    softmax on 128 lanes, alpha folded in (B = alpha*attn [+ I]), 32x32 block
    transpose -> B^T per partition group.
  Phase 2 (per slot): TensorE computes alpha*attn @ q in fp32r with 4
    concurrent 32x32 tiles (tile_position); the residual is added exactly in
    fp32 by DVE tensor_add from PSUM onto qt ("exact" mode; "fused" mode
    folds I into B and uses a single ScalarE copy instead); the slot is
    DMA'd to HBM (y kept in the folded layout, de-folded on host).
"""

import sys

sys.path.insert(0, "/opt/trn_rl_repo")

from contextlib import ExitStack

import numpy as np

import concourse.bass as bass
import concourse.tile as tile
from concourse import bacc, mybir

# Problem shape (hardcoded per contract)
N, C, T, H, W = 8, 128, 32, 28, 28
HB = H * W  # 784
F = T * HB  # 25088
G = 4  # partition groups (c blocks of 32)
CL = 32  # c-local within group
NCORES = 8

f32 = mybir.dt.float32
f32r = mybir.dt.float32r
bf16 = mybir.dt.bfloat16
AF = mybir.ActivationFunctionType
ALU = mybir.AluOpType
AX = mybir.AxisListType


def build_nc(
    mode: str = "exact",  # "exact" | "fused"
    nslot: int = 4,  # hw-striped chunks/slots
    nmm: int = 392,  # matmul2 moving free size (>=256 for fp32r fast path)
    cast_sub: int = 4,  # cast pieces per chunk
    gs_num: int = 0,  # of every gs_den evac groups, this many go via GpSimd
    gs_den: int = 2,
    mm2_dtype: str = "bf16",  # "bf16" | "f32"
    stores_per_slot: int = 1,  # 1, 2 or 4 store DMAs per slot
    energy_pack: int = 1,  # hw columns per energy matmul (1 or 2)
):
    assert HB % nslot == 0
    Js = HB // nslot  # hw per chunk/slot
    SW = Js * CL  # slot logical width (6272 for nslot=4)
    assert SW % nmm == 0
    nk = SW // nmm  # mm chunks per slot
    assert nk % 4 == 0
    assert CL % (2 * stores_per_slot) == 0

    nc = bacc.Bacc(trn_type="TRN2", target_bir_lowering=False, debug=False)

    x = nc.declare_dram_parameter("x", [C, F], f32, isOutput=False)
    al = nc.declare_dram_parameter("alpha_rep", [C, 1], f32, isOutput=False)
    i4 = nc.declare_dram_parameter("i4", [T, C], f32, isOutput=False)
    id32 = nc.declare_dram_parameter("ident32", [C, T], f32, isOutput=False)
    # y stored folded: y[32g+t, cl*HB+hw] = out[32g+cl, t, hw]; host de-folds.
    y = nc.declare_dram_parameter("y", [C, F], f32, isOutput=True)

    with ExitStack() as ctx:
        tc = ctx.enter_context(tile.TileContext(nc))
        consts = ctx.enter_context(tc.tile_pool(name="consts", bufs=1))
        smalls = ctx.enter_context(tc.tile_pool(name="smalls", bufs=1))
        xn_pool = ctx.enter_context(tc.tile_pool(name="xn", bufs=1))

        alpha_sb = consts.tile([C, 1], f32)
        nc.sync.dma_start(alpha_sb[:], al[:])
        i4_sb = consts.tile([T, C], f32)
        nc.sync.dma_start(i4_sb[:], i4[:])
        id_sb = consts.tile([C, T], f32)
        nc.sync.dma_start(id_sb[:], id32[:])
        # Warm the Exp activation table early (overlaps with phase-1 DMA).
        warm = consts.tile([C, 1], f32)
        nc.scalar.activation(warm[:], alpha_sb[:], AF.Exp)

        # XNQ = x (native) in cols [0, F) + one spare slot region at [F, F+SW)
        XNQ = xn_pool.tile([C, F + SW], f32)
        xn3 = XNQ[:, 0:F].rearrange("p (t h) -> p t h", t=T)  # [p][t][h]
        xn_hwT = XNQ[:, 0:F].rearrange("p (t h) -> p h t", t=T)  # [p][h][t]
        xq_clT = XNQ[:, 0:F].rearrange("p (cl h) -> p h cl", cl=CL)  # [p][h][cl]
        xd3 = x[:].rearrange("p (t h) -> p t h", t=T)

        # qt slot m lives at (base, cl-stride): slot 0 -> spare region
        # (compact, stride Js), slot m>=1 -> region m-1 (stride HB).
        def qt_cells(m, cl0, ncl, j0, nj, jmajor=False):
            """AP over qt slot m cells cl in [cl0, cl0+ncl), jj in [j0, j0+nj).
            Returns [p][cl][jj] (or [p][jj][cl] if jmajor)."""
            if m == 0:
                v = XNQ[:, F : F + SW].rearrange("p (cl j) -> p cl j", cl=CL)
                v = v[:, cl0 : cl0 + ncl, j0 : j0 + nj]
            else:
                base = (m - 1) * Js
                v = XNQ[:, 0:F].rearrange("p (cl h) -> p cl h", cl=CL)
                v = v[:, cl0 : cl0 + ncl, base + j0 : base + j0 + nj]
            if jmajor:
                s = "p cl j -> p j cl"
                v = v.rearrange(s)
            return v

        Bt = smalls.tile([C, T], f32)  # B^T, replicated per partition group

        with (
            tc.tile_pool(name="xnbf", bufs=1) as xnbf_pool,
            tc.tile_pool(name="psE", bufs=1, space="PSUM") as psE,
        ):
            XNbf = xnbf_pool.tile([C, F], bf16)
            xb3 = XNbf[:].rearrange("p (t h) -> p t h", t=T)
            xb_h2 = XNbf[:].rearrange("p (t h2 e) -> p t h2 e", t=T, e=energy_pack)
            E_ps = psE.tile([T * energy_pack, T * energy_pack], f32)

            # ---- Phase 1: load + cast + energy + transpose-to-folded ----
            for m in range(nslot):
                sl = slice(m * Js, (m + 1) * Js)
                nc.sync.dma_start(xn3[:, :, sl], xd3[:, :, sl])
                sub = Js // cast_sub
                for s in range(cast_sub):
                    lo = m * Js + s * sub
                    hi = lo + sub
                    nc.scalar.copy(xb3[:, :, lo:hi], xn3[:, :, lo:hi])
                    ne = sub // energy_pack
                    e0 = lo // energy_pack
                    for j in range(e0, e0 + ne):
                        a = xb_h2[:, :, j, :]
                        if energy_pack > 1:
                            a = a.rearrange("p t e -> p (e t)")
                        nc.tensor.matmul(
                            E_ps[:],
                            a,
                            a,
                            start=(j == 0),
                            stop=(j == HB // energy_pack - 1),
                        )
                # Fences: the StreamTranspose ISA struct has too few sync-wait
                # slots, so absorb its cross-engine deps (chunk-m DMA, chunk
                # m-1 ScalarE casts) into cheap DVE copies first; the
                # transpose then only needs its same-engine wait.
                fence = smalls.tile([C, 1], f32, tag="fence")
                nc.vector.tensor_copy(fence[:], xn3[:, 0:1, m * Js])
                if m >= 1:
                    # one element from each of chunk m-1's cast pieces
                    xb5 = XNbf[:].rearrange(
                        "p (t mm s o) -> p t mm s o", t=T, mm=nslot, s=cast_sub
                    )
                    fence2 = smalls.tile([C, cast_sub], f32, tag="fence2")
                    nc.vector.tensor_copy(
                        fence2[:].rearrange("p (a s c) -> p a s c", a=1, s=cast_sub),
                        xb5[:, 0:1, m - 1, :, 0:1],
                    )
                # transpose chunk m into qt slot m (region m-1 / spare)
                tin = xn_hwT[:, sl, :]
                tout = qt_cells(m, 0, CL, 0, Js, jmajor=True)
                nc.vector.transpose(tout, tin)

            # ---- Softmax -> B^T (replicated x4 on partition groups) ----
            assert energy_pack == 1  # pack=2 needs selector-matmul diag fold
            E_sb = smalls.tile([T, T], f32)
            nc.scalar.copy(E_sb[:], E_ps[:])
            Erep = psE.tile([C, T], f32)
            nc.tensor.matmul(Erep[:], i4_sb[:], E_sb[:], start=True, stop=True)
            negmax = smalls.tile([C, 1], f32)
            nc.vector.tensor_reduce(
                negmax[:], Erep[:], axis=AX.X, op=ALU.max, negate=True
            )
            P = smalls.tile([C, T], f32)
            nc.scalar.activation(P[:], Erep[:], AF.Exp, bias=negmax[:], scale=1.0)
            ssum = smalls.tile([C, 1], f32)
            nc.vector.tensor_reduce(ssum[:], P[:], axis=AX.X, op=ALU.add)
            rcp = smalls.tile([C, 1], f32)
            nc.vector.reciprocal(rcp[:], ssum[:])
            r2 = smalls.tile([C, 1], f32)
            nc.vector.tensor_mul(r2[:], rcp[:], alpha_sb[:])
            Bp = smalls.tile([C, T], f32)
            nc.vector.tensor_scalar_mul(Bp[:], P[:], r2[:])
            if mode == "fused":
                nc.vector.tensor_add(Bp[:], Bp[:], id_sb[:])
            nc.vector.transpose(Bt[:], Bp[:])

        if mm2_dtype == "f32":
            BtT = Bt[:]
        else:
            Btb = smalls.tile([C, T], bf16)
            nc.vector.tensor_copy(Btb[:], Bt[:])
            BtT = Btb[:]

        # ---- Phase 2: attention matmul + residual + store ----
        y3 = y[:].rearrange("p (cl h) -> p cl h", cl=CL)
        ncl_mm = nmm // Js  # cl columns per matmul chunk
        with (
            tc.tile_pool(name="tmp", bufs=2) as tmpp,
            tc.tile_pool(name="ps2", bufs=2, space="PSUM") as ps2,
        ):
            evac_idx = 0
            for m in range(nslot):
                for k in range(nk // 4):
                    if mm2_dtype == "bf16":
                        # ScalarE casts this 4-chunk group of folded q to bf16
                        qtb = tmpp.tile([C, 4 * nmm], bf16, tag="qtb")
                        qb4 = qtb[:].rearrange(
                            "p (b cl2 j) -> p b cl2 j", b=4, cl2=ncl_mm
                        )
                        src = qt_cells(m, k * 4 * ncl_mm, 4 * ncl_mm, 0, Js).rearrange(
                            "p (b cl2) j -> p b cl2 j", b=4
                        )
                        nc.scalar.copy(qb4, src)
                    ps = ps2.tile([C, 2048], f32)
                    for b in range(4):
                        ck = k * 4 + b
                        for g in range(G):
                            if mm2_dtype == "bf16":
                                rv = qtb[
                                    g * 32 : (g + 1) * 32,
                                    b * nmm : (b + 1) * nmm,
                                ]
                            else:
                                rv = qt_cells(m, ck * ncl_mm, ncl_mm, 0, Js)[
                                    g * 32 : (g + 1) * 32
                                ]
                            nc.tensor.matmul(
                                ps[g * 32 : (g + 1) * 32, b * 512 : b * 512 + nmm],
                                BtT[g * 32 : (g + 1) * 32, :],
                                rv,
                                start=True,
                                stop=True,
                                tile_position=(g * 32, g * 32),
                            )
                    # psum bank b cols [0, nmm) hold (cl2, j) for chunk 4k+b
                    pv = (
                        ps[:]
                        .rearrange("p (b r) -> p b r", b=4)[:, :, 0:nmm]
                        .rearrange("p b (cl2 j) -> p b cl2 j", cl2=ncl_mm)
                    )
                    ncl_g = 4 * ncl_mm
                    qv = qt_cells(m, k * ncl_g, ncl_g, 0, Js).rearrange(
                        "p (b cl2) j -> p b cl2 j", b=4
                    )
                    if mode == "fused":
                        nc.scalar.copy(qv, pv)
                    else:
                        use_gp = (evac_idx % gs_den) < gs_num
                        evac_idx += 1
                        if use_gp:
                            tmp = tmpp.tile([C, 4 * nmm], f32, tag="evac")
                            t3 = tmp[:].rearrange(
                                "p (b cl2 j) -> p b cl2 j", b=4, cl2=ncl_mm
                            )
                            nc.scalar.copy(t3, pv)
                            nc.gpsimd.tensor_add(qv, qv, t3)
                        else:
                            nc.vector.tensor_add(qv, qv, pv)
                # store slot (128-partition, 3-dim DMAs into folded y)
                ncl_st = CL // stores_per_slot
                for s in range(stores_per_slot):
                    sb = qt_cells(m, s * ncl_st, ncl_st, 0, Js)
                    dr = y3[:, s * ncl_st : (s + 1) * ncl_st, m * Js : (m + 1) * Js]
                    nc.sync.dma_start(dr, sb)

    nc.compile()  # bacc passes: reg alloc, wait splitting (1-wait HW limit), ...
    return nc


def _consts():
    i4 = np.zeros((T, C), np.float32)
    for mcol in range(C):
        i4[mcol % T, mcol] = 1.0
    id32 = np.zeros((C, T), np.float32)
    for p in range(C):
        id32[p, p % T] = 1.0
    return i4, id32


_BUILD_KW = dict(mode="exact")


def kernel(x: np.ndarray, alpha: np.ndarray) -> np.ndarray:
    from concourse.bass_utils import run_bass_kernel_spmd

    assert x.shape == (N, C, T, H, W) and x.dtype == np.float32
    nc = build_nc(**_BUILD_KW)
    i4, id32 = _consts()
    alpha_rep = np.full((C, 1), np.float32(alpha.reshape(-1)[0]), np.float32)
    xr = np.ascontiguousarray(x.reshape(N, C, F))
    in_maps = [
        {"x": xr[n], "alpha_rep": alpha_rep, "i4": i4, "ident32": id32}
        for n in range(NCORES)
    ]
    res = run_bass_kernel_spmd(nc, in_maps, list(range(NCORES)))
    out = np.stack([unfold_y(res.results[n]["y"]) for n in range(NCORES)])
    return out.astype(np.float32)


def unfold_y(yf: np.ndarray) -> np.ndarray:
    # yf[32g+t, cl*HB+hw] = out[32g+cl, t, hw]  ->  (C, T, H, W)
    return (
        np.asarray(yf)
        .reshape(G, T, CL, HB)
        .transpose(0, 2, 1, 3)
        .reshape(C, T, H, W)
    )
